# revision 15
# baseline (speedup 1.0000x reference)
"""Trainium2 Bass kernel for nn_Block_60601988546807.

Per-sample modulated 3x3 conv + GroupNorm(8) + SiLU, data-parallel over
batch: 16 samples -> 8 NeuronCores, 2 samples/core.

Per core, per sample:
  1. weight prep (tiny): softmax over kernel_mod, blend 4 weight banks,
     PE-transpose to [c_in, 9, c_out], modulate by (mod+1); demod norm is
     folded into the conv epilogue (linearity).
  2. conv: 9 shifted float32r matmuls (K=64,M=64,N=512) accumulated in PSUM
     over a width-padded x window; evacuate PSUM*inv_norm -> fp16 y buffer
     in SBUF, with per-channel sum/sumsq accumulated on the fly (accum_out).
  3. GroupNorm stats: per-group reduction via a tiny indicator matmul;
     pass 2 = one ACT op per chunk: Silu(scale*y + shift), DMA out.
"""
import sys
sys.path.insert(0, '/opt/trn_rl_repo')

import numpy as np
import concourse.bass as bass
import concourse.tile as tile
from concourse import mybir
from concourse.bass_utils import run_bass_kernel_spmd

F32 = mybir.dt.float32
F32R = mybir.dt.float32r
F16 = mybir.dt.float16
AF = mybir.ActivationFunctionType
ALU = mybir.AluOpType

B = 2            # samples per core
C = 64           # channels in/out
H = W = 256
WP = W + 2       # width-padded row
NK = 4           # conv kernel banks
G = 8            # groupnorm groups
BR = 16          # output rows per x block
NBLK = 128 // BR # blocks per half
NPIX = (C // G) * H * W  # elements per group
EPS_DEMOD = 1e-8
EPS_GN = 1e-5
N_CORES = 8


def _split_multiwait(nc, max_waits=1):
    """This walrus build encodes at most one semaphore wait per instruction.
    Move excess waits onto preceding single-wait NoOps on the same engine
    (equivalent: engines execute their stream in order)."""
    for fn in nc.m.functions:
        for bb in fn.blocks:
            newlist = []
            for inst in bb.instructions:
                si = inst.sync_info
                if si is not None and si.on_wait and len(si.on_wait) > max_waits:
                    waits = list(si.on_wait)
                    for j, wcond in enumerate(waits[max_waits:]):
                        nop = mybir.InstNoOp(
                            name=f"{inst.name}-ws{j}", engine=inst.engine,
                            ins=[], outs=[],
                            sync_info=mybir.SyncInfo(on_wait=[wcond], on_update=[]),
                        )
                        nc.register_instruction(nop)
                        newlist.append(nop)
                    si.on_wait = waits[:max_waits]
                newlist.append(inst)
            bb.instructions = newlist


def build_nc():
    nc = bass.Bass(dynamic_dma_scratch_size=8192)
    x_ext = nc.declare_dram_parameter("x", [B, C, H, W], F32R, isOutput=False)
    mod_ext = nc.declare_dram_parameter("mod", [B, C], F32, isOutput=False)
    km_ext = nc.declare_dram_parameter("km", [B, NK], F32, isOutput=False)
    cw_ext = nc.declare_dram_parameter("cw", [NK, C, C, 9], F32, isOutput=False)
    gam_ext = nc.declare_dram_parameter("gam", [C], F32, isOutput=False)
    bet_ext = nc.declare_dram_parameter("bet", [C], F32, isOutput=False)
    cst_ext = nc.declare_dram_parameter("cst", [C, 2 * C], F32, isOutput=False)
    y_ext = nc.declare_dram_parameter("y", [B, C, H, W], F32, isOutput=True)

    from contextlib import ExitStack
    with tile.TileContext(nc) as tc, ExitStack() as ctx:
        persist = ctx.enter_context(tc.tile_pool(name="persist", bufs=1))
        wpool = ctx.enter_context(tc.tile_pool(name="wpool", bufs=2))
        xpool = ctx.enter_context(tc.tile_pool(name="xpool", bufs=2))
        ypool = ctx.enter_context(tc.tile_pool(name="ypool", bufs=1))
        opool = ctx.enter_context(tc.tile_pool(name="opool", bufs=3))
        pp = ctx.enter_context(tc.tile_pool(name="pp", bufs=4, space="PSUM"))
        pt = ctx.enter_context(tc.tile_pool(name="pt", bufs=2, space="PSUM"))

        # ---- one-time loads ----
        consts = persist.tile([C, 2 * C], F32, name="consts")
        nc.sync.dma_start(out=consts[:], in_=cst_ext[:])
        ident = consts[:, 0:C]
        gmat = consts[:, C:2 * C]

        w_nat = persist.tile([C, NK, C * 9], F32, name="w_nat")  # [o, n, (i kl)]
        nc.sync.dma_start(out=w_nat[:], in_=cw_ext.rearrange("n o i k -> o n (i k)"))

        gam_t = persist.tile([C, 1], F32, name="gam_t")
        bet_t = persist.tile([C, 1], F32, name="bet_t")
        nc.gpsimd.dma_start(out=gam_t[:], in_=gam_ext.rearrange("(c u) -> c u", u=1))
        nc.gpsimd.dma_start(out=bet_t[:], in_=bet_ext.rearrange("(c u) -> c u", u=1))

        ones_c = persist.tile([C, 1], F32, name="ones_c")
        nc.vector.memset(ones_c[:], 1.0)
        ones_r = persist.tile([1, C], F32, name="ones_r")
        nc.vector.memset(ones_r[:], 1.0)

        # ---- softmax over kernel_mod, on partition 0: [1, B*NK] ----
        km_t = persist.tile([1, B * NK], F32, name="km_t")
        nc.gpsimd.dma_start(out=km_t[:], in_=km_ext.rearrange("b n -> (b n)").rearrange("(u q) -> u q", u=1))
        attn_r = persist.tile([1, B * NK], F32, name="attn_r")
        for s in range(B):
            sl = km_t[:, s * NK:(s + 1) * NK]
            mx = persist.tile([1, 1], F32, name=f"mx{s}")
            nc.vector.reduce_max(mx[:], sl, axis=mybir.AxisListType.X)
            ex = persist.tile([1, NK], F32, name=f"ex{s}")
            nc.vector.tensor_scalar(out=ex[:], in0=sl, scalar1=mx[:],
                                    scalar2=None, op0=ALU.subtract)
            nc.scalar.activation(out=ex[:], in_=ex[:], func=AF.Exp)
            sm = persist.tile([1, 1], F32, name=f"sm{s}")
            nc.vector.reduce_sum(sm[:], ex[:], axis=mybir.AxisListType.X)
            nc.vector.reciprocal(sm[:], sm[:])
            nc.vector.tensor_scalar_mul(out=attn_r[:, s * NK:(s + 1) * NK],
                                        in0=ex[:], scalar1=sm[:])
        # broadcast partition 0 -> 64 partitions via ones-matmul
        attn_b = persist.tile([C, B, NK], F32, name="attn_b")
        ps_a = pt.tile([C, B * NK], F32, name="ps_a", tag="ps_d")
        nc.tensor.matmul(ps_a[:], ones_r[:], attn_r[:], start=True, stop=True)
        nc.vector.tensor_copy(attn_b[:].rearrange("c b n -> c (b n)"), ps_a[:])

        for s in range(B):
            # ================= weight prep =================
            w_blend = wpool.tile([C, C * 9], F32, name=f"w_blend{s}", tag="w_blend")
            nc.vector.tensor_scalar_mul(out=w_blend[:], in0=w_nat[:, 0, :],
                                        scalar1=attn_b[:, s, 0:1])
            for n in range(1, NK):
                nc.vector.scalar_tensor_tensor(
                    out=w_blend[:], in0=w_nat[:, n, :], scalar=attn_b[:, s, n:n + 1],
                    in1=w_blend[:], op0=ALU.mult, op1=ALU.add)

            # transpose [o,(i kl)] -> w_t [i, kl, o]
            w_t = wpool.tile([C, 9, C], F32, name=f"w_t{s}", tag="w_t")
            wb_v = w_blend[:].rearrange("o (i k) -> o k i", k=9)
            for kl in range(9):
                ps_t = pt.tile([C, C], F32, name=f"ps_t{s}_{kl}", tag="ps_t")
                nc.tensor.transpose(ps_t[:], wb_v[:, kl, :], ident)
                nc.vector.tensor_copy(w_t[:, kl, :], ps_t[:])

            # modulate by (mod+1) along input channel (partition dim)
            modp1 = wpool.tile([C, 1], F32, name=f"modp1{s}", tag="modp1")
            nc.gpsimd.dma_start(out=modp1[:], in_=mod_ext[s].rearrange("(c u) -> c u", u=1))
            nc.scalar.add(out=modp1[:], in_=modp1[:], add=1.0)
            w_tv = w_t[:].rearrange("i k o -> i (k o)")
            nc.vector.tensor_scalar_mul(out=w_tv, in0=w_tv, scalar1=modp1[:])
            w_tr = wpool.tile([C, 9, C], F32R, name=f"w_tr{s}", tag="w_tr")
            nc.vector.tensor_copy(w_tr[:].rearrange("i k o -> i (k o)"), w_tv)

            # demod: inv[o] = rsqrt(clip(sum_{i,kl} w^2, 1e-8))
            w2 = wpool.tile([C, 9, C], F32, name=f"w2_{s}", tag="w2")
            nc.vector.tensor_mul(w2[:].rearrange("i k o -> i (k o)"), w_tv, w_tv)
            sumk = wpool.tile([C, C], F32, name=f"sumk{s}", tag="sumk")
            # reduce over kl: view [i, o, kl] so kl is innermost
            nc.vector.tensor_reduce(sumk[:], w2[:].rearrange("i k o -> i o k"),
                                    axis=mybir.AxisListType.X, op=ALU.add)
            ps_d = pt.tile([1, C], F32, name=f"ps_d{s}", tag="ps_d")
            nc.tensor.matmul(ps_d[:], ones_c[:], sumk[:], start=True, stop=True)
            inv_r = wpool.tile([1, C], F32, name=f"inv_r{s}", tag="inv_r")
            nc.vector.tensor_scalar_max(out=inv_r[:], in0=ps_d[:], scalar1=EPS_DEMOD)
            nc.scalar.activation(out=inv_r[:], in_=inv_r[:], func=AF.Sqrt)
            nc.vector.reciprocal(inv_r[:], inv_r[:])
            ps_i = pt.tile([C, 1], F32, name=f"ps_i{s}", tag="ps_t")
            nc.tensor.transpose(ps_i[:], inv_r[:], ident[0:1, 0:1])
            inv_c = wpool.tile([C, 1], F32, name=f"inv_c{s}", tag="inv_c")
            nc.vector.tensor_copy(inv_c[:], ps_i[:])

            # ================= conv + stats =================
            y_sb = ypool.tile([C, H, W], F16, name=f"y_sb{s}", tag="y_sb")
            sums = wpool.tile([C, 2 * NBLK * (BR // 2), 2], F32,
                              name=f"sums{s}", tag="sums")  # [c, tile, {sum,sumsq}]
            sq_scr = opool.tile([C, 2, W], F16, name=f"sq_scr{s}", tag="sq_scr", bufs=1)

            for half in range(2):
                for blk in range(NBLK):
                    r0 = half * 128 + blk * BR          # first output row
                    in_lo, in_hi = r0 - 1, r0 + BR      # input rows inclusive
                    lo, hi = max(in_lo, 0), min(in_hi, H - 1)
                    slot = lo - in_lo
                    x_t = xpool.tile([C, BR + 2, WP], F32R, name=f"x_{s}_{half}_{blk}",
                                     tag="x_t")
                    nc.gpsimd.memset(x_t[:, :, 0:1].bitcast(F32), 0.0)
                    nc.gpsimd.memset(x_t[:, :, W + 1:W + 2].bitcast(F32), 0.0)
                    if in_lo < 0:
                        nc.gpsimd.memset(x_t[:, 0:1, :].bitcast(F32), 0.0)
                    if in_hi > H - 1:
                        nc.gpsimd.memset(x_t[:, BR + 1:BR + 2, :].bitcast(F32), 0.0)
                    nc.sync.dma_start(out=x_t[:, slot:slot + hi - lo + 1, 1:W + 1],
                                      in_=x_ext[s, :, lo:hi + 1, :])
                    for t in range(BR // 2):
                        ps = pp.tile([C, 2, W], F32, name=f"ps{s}_{half}_{blk}_{t}",
                                     tag="ps")
                        for tap in range(9):
                            ky, kx = tap // 3, tap % 3
                            rhs = x_t[:, t * 2 + ky:t * 2 + ky + 2, kx:kx + W]
                            nc.tensor.matmul(ps[:].rearrange("c a b -> c (a b)"),
                                             w_tr[:, tap, :], rhs,
                                             start=(tap == 0), stop=(tap == 8))
                        idx = half * 64 + blk * (BR // 2) + t
                        r = half * 128 + blk * BR + t * 2
                        dst = y_sb[:, r:r + 2, :]
                        nc.vector.tensor_scalar(
                            out=dst, in0=ps[:], scalar1=inv_c[:], scalar2=0.0,
                            op0=ALU.mult, op1=ALU.add,
                            accum_out=sums[:, idx, 0:1])
                        nc.scalar.activation(
                            out=sq_scr[:], in_=dst, func=AF.Square,
                            accum_out=sums[:, idx, 1:2])

            # ================= groupnorm scalars =================
            st2 = wpool.tile([C, 2], F32, name=f"st2_{s}", tag="st2")
            nc.vector.tensor_reduce(st2[:], sums[:].rearrange("c t u -> c u t"),
                                    axis=mybir.AxisListType.X, op=ALU.add)
            ps_g = pt.tile([C, 2], F32, name=f"ps_g{s}", tag="ps_d")
            nc.tensor.matmul(ps_g[:], gmat, st2[:], start=True, stop=True)
            mean_c = wpool.tile([C, 1], F32, name=f"mean{s}", tag="mean")
            nc.scalar.mul(out=mean_c[:], in_=ps_g[:, 0:1], mul=1.0 / NPIX)
            m2 = wpool.tile([C, 1], F32, name=f"m2_{s}", tag="m2")
            nc.vector.tensor_mul(m2[:], mean_c[:], mean_c[:])
            var_c = wpool.tile([C, 1], F32, name=f"var{s}", tag="var")
            nc.vector.scalar_tensor_tensor(out=var_c[:], in0=ps_g[:, 1:2],
                                           scalar=1.0 / NPIX, in1=m2[:],
                                           op0=ALU.mult, op1=ALU.subtract)
            eps_t = wpool.tile([C, 1], F32, name=f"eps{s}", tag="eps")
            nc.vector.memset(eps_t[:], EPS_GN)
            rstd = wpool.tile([C, 1], F32, name=f"rstd{s}", tag="rstd")
            nc.scalar.activation(out=rstd[:], in_=var_c[:], func=AF.Sqrt,
                                 bias=eps_t[:], scale=1.0)
            nc.vector.reciprocal(rstd[:], rstd[:])
            scale_c = wpool.tile([C, 1], F32, name=f"scale{s}", tag="scale")
            nc.vector.tensor_mul(scale_c[:], rstd[:], gam_t[:])
            shift_c = wpool.tile([C, 1], F32, name=f"shift{s}", tag="shift")
            nc.vector.tensor_mul(shift_c[:], mean_c[:], scale_c[:])
            nc.vector.tensor_tensor(out=shift_c[:], in0=bet_t[:], in1=shift_c[:],
                                    op=ALU.subtract)

            # ================= pass 2: Silu(scale*y + shift) =================
            CS = 1024
            y_flat = y_sb[:].rearrange("c h w -> c (h w)")
            y_oflat = y_ext[s].rearrange("c h w -> c (h w)")
            for k in range(H * W // CS):
                o_t = opool.tile([C, CS], F32, name=f"o_{s}_{k}", tag="o_t")
                nc.scalar.activation(out=o_t[:], in_=y_flat[:, k * CS:(k + 1) * CS],
                                     func=AF.Silu, scale=scale_c[:], bias=shift_c[:])
                nc.sync.dma_start(out=y_oflat[:, k * CS:(k + 1) * CS], in_=o_t[:])

    _split_multiwait(nc)
    return nc


_NC = None


def kernel(x, mod, kernel_mod, conv_weights, gn_gamma, gn_beta):
    global _NC
    if _NC is None:
        _NC = build_nc()
    nc = _NC
    bsz = x.shape[0]
    per = bsz // N_CORES
    cst = np.concatenate([np.eye(C, dtype=np.float32),
                          np.kron(np.eye(G), np.ones((C // G, C // G))).astype(np.float32)],
                         axis=1)
    cw = np.ascontiguousarray(conv_weights.reshape(NK, C, C, 9), dtype=np.float32)
    in_maps = []
    for i in range(N_CORES):
        sl = slice(i * per, (i + 1) * per)
        in_maps.append({
            "x": np.ascontiguousarray(x[sl], dtype=np.float32),
            "mod": np.ascontiguousarray(mod[sl], dtype=np.float32),
            "km": np.ascontiguousarray(kernel_mod[sl], dtype=np.float32),
            "cw": cw,
            "gam": np.ascontiguousarray(gn_gamma, dtype=np.float32),
            "bet": np.ascontiguousarray(gn_beta, dtype=np.float32),
            "cst": cst,
        })
    global _last_in_maps
    _last_in_maps = in_maps
    res = run_bass_kernel_spmd(nc, in_maps, core_ids=list(range(N_CORES)))
    return np.concatenate([r["y"] for r in res.results], axis=0)


_last_in_maps = None

